# revision 1
# baseline (speedup 1.0000x reference)
"""CRF Viterbi decode kernel for Trainium2 (8 NeuronCores, pure data parallel).

Problem: X [4096, 512, 128] f32, W [26, 128], T [26, 26]
  e = einsum('bld,kd->blk', X, W)
  per word: Viterbi max-sum scan over L=512 with transition T, backtrace,
  output int32 labels [4096, 512].

Strategy (per core, 512 words):
  - shard batch across 8 cores (512 words each), replicate W/T.
  - words on partitions: 4 groups of 128 words.
  - emissions on PE: per (group, position) transpose X tile [128w,128d] ->
    [128d,128w] (PE transpose), matmul with W^T -> e [128w, 26] in PSUM,
    batch-copy to SBUF, spill e to DRAM (needed again by backward scan).
  - forward scan (l) and backward scan (G) instead of backtrace:
      l[i,y] = max_j(e[i-1,j] + T[j,y] + l[i-1,j])     (reference's scan)
      G[i,j] = e[i,j] + max_y(T[j,y] + G[i+1,y]),  G[L-1] = e[L-1]
      label[i] = argmax_y(l[i,y] + G[i,y])
    We keep m = e + l (the quantity the scan consumes anyway); then
    t[i] = m[i] + maxpart[i] where G[i] = e[i] + maxpart[i].
  - the inner max-plus step uses a hand-built custom DVE op SEG_MAX_ADD_ANT:
    out[p,s,k] = running max over k within page s of (in0+in1), so
    out[:, y, 25] = max_j(m[j] + T[j,y]) in ONE pass (fused add+max with
    per-page reset via the SUB_DIM_DONE step-state, like TENSOR_PAGED_MASK).
  - m is spilled to DRAM during the forward pass, streamed back (with e)
    during the backward pass; labels via bulk masked argmax per chunk.
"""

import os
import sys

for _p in ("/opt/trn_rl_repo", "/root/.axon_site/_ro/trn_rl_repo"):
    if os.path.isdir(_p) and _p not in sys.path:
        sys.path.append(_p)

import dataclasses
import numpy as np

import concourse.bass as bass
import concourse.tile as tile
from concourse import bacc, mybir
from concourse.bass_utils import run_bass_kernel_spmd

F32 = mybir.dt.float32
I32 = mybir.dt.int32

NUM_CORES = 8
K = 26
D = 128
NG = 4  # word groups of 128 per core

# ---------------------------------------------------------------------------
# Custom DVE op: segmented (per-page) running max of (Src0 + Src1).
# ---------------------------------------------------------------------------

_SEGMAX_NAME = "SEG_MAX_ADD_ANT"
_segmax_op = None


def _segmax_reference(in0, in1, c0, c1, c2):
    z = np.asarray(in0, np.float32) + np.asarray(in1, np.float32)
    return np.maximum.accumulate(z, axis=-1)


def _build_segmax_uops(ver):
    from concourse import dve_spec as ds
    from concourse.dve_spec import AluOp, Spec, Src0, Src1, Trigger, scan

    spec = Spec(body=scan(AluOp.MAX, Src0 + Src1), reference=_segmax_reference)
    ds._validate_body(spec, ver)
    spec2 = ds._hoist_stream_invariant_ops(spec)
    scans = ds._collect(spec2.body, ds.Scan)
    placement = ds._build_placement(spec2, scans, ds.N_STAGES[ver], ds.N_LANES[ver])
    base = ds._build_state_machine(spec2, scans, [], placement)
    assert len(base) == 2  # [seed, steady]
    d = placement.node_stage[scans[0]]
    steady_stage = placement.pipeline[d]
    seed = base[0]
    steady = dataclasses.replace(
        base[1],
        trigger=(Trigger.SRC_TENSOR_DONE, Trigger.SUB_DIM_DONE, Trigger.NONE),
        next=(0, 2, 0),
    )
    # Boundary element: recombine against -FLT_MAX instead of the running
    # value -> the fold restarts at each page, exactly the PageIdx step-state
    # shape with the combine kept.
    step = dataclasses.replace(
        base[1],
        trigger=(Trigger.SRC_TENSOR_DONE, Trigger.SUB_DIM_DONE, Trigger.COUNT),
        next=(0, 2, 1),
        repeat=1,
        overrides={d: ds._Stage(AluOp.MAX, ds.MaxNeg, steady_stage.b)},
    )
    uops = [ds._assemble(s) for s in (seed, steady, step)]
    for u in uops:
        u.validate(ver)
    return spec, uops


def get_segmax_op():
    """Build + register the custom op in the dve_ops registries (idempotent)."""
    global _segmax_op
    if _segmax_op is not None:
        return _segmax_op
    from concourse import dve_ops
    from concourse.dve_ops import OPS, CUSTOM_DVE_SPECS, _SUB_OPCODE_FOR_NAME, DveOp
    from concourse.dve_uop import DveOpSpec

    @dataclasses.dataclass(frozen=True)
    class _HandBuiltDveOp(DveOp):
        def compile(self, ver):
            key = (self.name, ver)
            if (r := dve_ops._COMPILE_CACHE.get(key)) is not None:
                return r
            from concourse.dve_ops import get_dve_sub_opcode

            _, uops = _build_segmax_uops(ver)
            result = DveOpSpec(
                name=self.name,
                opcode=get_dve_sub_opcode(self.name),
                uops=uops,
                rd1_en=True,
            )
            dve_ops._COMPILE_CACHE[key] = result
            return result

    spec, _ = _build_segmax_uops("v3")
    op = _HandBuiltDveOp(_SEGMAX_NAME, spec, subdim=True, uops_sha={})
    if _SEGMAX_NAME not in _SUB_OPCODE_FOR_NAME:
        OPS.append(op)
        CUSTOM_DVE_SPECS[_SEGMAX_NAME] = spec
        _SUB_OPCODE_FOR_NAME[_SEGMAX_NAME] = 1 + len(OPS) - 1
        assert _SUB_OPCODE_FOR_NAME[_SEGMAX_NAME] < 0x20
    _segmax_op = op
    return op


def _custom_dve_raw(vec, op, out, in0, in1):
    """_custom_dve minus the <=2-free-dim assert: emit one InstCustomDveAnt
    over 3-free-dim APs so all 4 word groups ride in a single instruction.
    SUB_DIM_DONE fires on every innermost-dim wrap, so per-(g,y)-page reset
    semantics are unchanged."""
    import concourse.bass_isa as bass_isa
    from concourse.bass import MemorySpace
    from concourse.dve_ops import get_dve_sub_opcode
    from concourse.dve_table_gen import dve_ver_for
    from concourse import mybir as mb

    nc = vec.bass
    if op.name not in nc.m.ant_custom_dve_ops:
        nc.m.ant_custom_dve_ops = sorted({*nc.m.ant_custom_dve_ops, op.name})
    compiled = op.compile(dve_ver_for(nc.trn_type))
    shape = bass_isa.CustomDveShape.STT  # in1 has 2+ free dims
    isa_opcode = nc.isa.Opcode[
        f"NEURON_ISA_TPB_OPCODE_CUSTOM_DVE_ANT_{shape.slot()}"
    ].value
    imm = mb.ImmediateValue(dtype=mb.dt.float32, value=0.0)
    return vec.add_instruction(
        bass_isa.InstCustomDveAnt(
            name=nc.get_next_instruction_name(),
            op_name=op.name,
            rd1_en=compiled.rd1_en,
            subdim=0x02 if op.subdim else 0,
            imm2=0.0,
            shape=shape,
            row=get_dve_sub_opcode(op.name),
            isa_opcode=isa_opcode,
            ins=[
                vec.lower_ap(in0, for_isa=True, opt=False),
                vec.lower_ap(in1, for_isa=True, opt=False),
                imm,
                mb.ImmediateValue(dtype=mb.dt.float32, value=0.0),
            ],
            outs=[vec.lower_ap(out, for_isa=True, opt=False)],
        )
    )


# ---------------------------------------------------------------------------
# Program builder
# ---------------------------------------------------------------------------


def build_crf_kernel(ctx, tc, out_aps, in_aps, L=512, use_fused=True, adds_engine="vector"):
    """Emit the per-core CRF decode program.

    in_aps: dict with DRAM APs: X [NG*128, L, D], wt [128, K] (= W^T),
      trepf [128, K*K] (T^T flat: [y*26+j] = T[j,y]),
      trepg [128, K*K] (T flat:  [j*26+y] = T[j,y]),
      ident [128, 128], revy [128, K] (= 26 - y).
    out_aps: dict with labels [NG*128, L] int32.
    """
    nc = tc.nc
    X = in_aps["X"]
    labels_out = out_aps["labels"]
    adds = nc.gpsimd if adds_engine == "gpsimd" else nc.vector

    KK = K * K
    CH = min(32, L)  # scan/spill chunk
    assert L % CH == 0
    NCH = L // CH
    XCH = min(8, L)  # X staging sub-chunk
    EPS = min(16, CH)  # emissions per PSUM bank tile (16*26=416 fp32 <= 512)

    segmax = get_segmax_op() if use_fused else None

    # DRAM scratch for e and m, layout [i, w, k] (i-major so a chunk of
    # positions for all words is one contiguous region).
    e_dram = nc.dram_tensor(f"e_scr_{L}", [L, NG * 128, K], F32).ap()
    m_dram = nc.dram_tensor(f"m_scr_{L}", [L, NG * 128, K], F32).ap()

    cpool = ctx.enter_context(tc.tile_pool(name="consts", bufs=1))
    trepf = cpool.tile([128, KK], F32, tag="trepf")
    trepg = cpool.tile([128, KK], F32, tag="trepg")
    wt = cpool.tile([128, K], F32, tag="wt")
    ident = cpool.tile([128, 128], F32, tag="ident")
    revy = cpool.tile([128, K], F32, tag="revy")
    nc.sync.dma_start(trepf[:], in_aps["trepf"])
    nc.sync.dma_start(trepg[:], in_aps["trepg"])
    nc.sync.dma_start(wt[:], in_aps["wt"])
    nc.sync.dma_start(ident[:], in_aps["ident"])
    nc.sync.dma_start(revy[:], in_aps["revy"])
    trepf3 = trepf[:].rearrange("p (y j) -> p y j", j=K)
    trepg3 = trepg[:].rearrange("p (j y) -> p j y", y=K)

    labels_pool = ctx.enter_context(tc.tile_pool(name="labels", bufs=1))
    labels_sb = labels_pool.tile([128, NG, L], I32)

    # ---------------- Phase A: emissions + forward scan ----------------
    with (
        tc.tile_pool(name="xstage", bufs=2) as pool_x,
        tc.tile_pool(name="esb", bufs=3) as pool_esb,
        tc.tile_pool(name="mout", bufs=3) as pool_m,
        tc.tile_pool(name="sall", bufs=3) as pool_s,
        tc.tile_pool(name="lscr", bufs=2) as pool_l,
        tc.tile_pool(name="xt_ps", bufs=3, space="PSUM") as pool_xtps,
        tc.tile_pool(name="eps", bufs=1, space="PSUM") as pool_eps,
        tc.tile_pool(name="xtsb", bufs=3) as pool_xtsb,
    ):
        m_prev = None
        for c in range(NCH):
            i0 = c * CH
            e_sb = pool_esb.tile([128, NG, CH, K], F32, tag="esb")
            m_c = pool_m.tile([128, CH, NG, K], F32, tag="mout")
            for g in range(NG):
                # emissions for (group g, positions i0..i0+CH)
                for h in range(CH // EPS):
                    eps = pool_eps.tile([128, EPS * K], F32, tag=f"eps{g}")
                    for ii in range(EPS):
                        pos = h * EPS + ii
                        sc, si = divmod(pos, XCH)
                        if si == 0:
                            xs = pool_x.tile([128, XCH * D], F32, tag=f"xs{g}")
                            nc.sync.dma_start(
                                xs[:],
                                X[g * 128 : (g + 1) * 128, i0 + sc * XCH : i0 + (sc + 1) * XCH, :],
                            )
                            xs3 = xs[:].rearrange("p (i d) -> p i d", d=D)
                        xt_ps = pool_xtps.tile([128, 128], F32, tag="xt")
                        nc.tensor.transpose(xt_ps[:], xs3[:, si, :], ident[:])
                        xt_sb = pool_xtsb.tile([128, 128], F32, tag="xt_sb")
                        nc.scalar.copy(xt_sb[:], xt_ps[:])
                        nc.tensor.matmul(
                            eps[:, ii * K : (ii + 1) * K],
                            lhsT=xt_sb[:],
                            rhs=wt[:],
                            start=True,
                            stop=True,
                            skip_group_check=True,
                        )
                    nc.scalar.copy(
                        e_sb[:, g, h * EPS : (h + 1) * EPS, :],
                        eps[:].rearrange("p (i k) -> p i k", k=K),
                    )
            for g in range(NG):
                nc.sync.dma_start(
                    e_dram[i0 : i0 + CH, g * 128 : (g + 1) * 128, :].rearrange(
                        "i p k -> p i k"
                    ),
                    e_sb[:, g, :, :],
                )
            # forward scan over this chunk
            for pos in range(CH):
                i = i0 + pos
                if i == 0:
                    nc.vector.tensor_copy(m_c[:, 0, :, :], e_sb[:, :, 0, :])
                    continue
                mp = m_prev[:, CH - 1, :, :] if pos == 0 else m_c[:, pos - 1, :, :]
                if use_fused == "fuse4":
                    s_all = pool_s.tile([128, NG, K, K], F32, tag="sall")
                    _custom_dve_raw(
                        nc.vector,
                        segmax,
                        out=s_all[:],
                        in0=mp.unsqueeze(2).broadcast_to([128, NG, K, K]),
                        in1=trepf3.unsqueeze(1).broadcast_to([128, NG, K, K]),
                    )
                    lpart = s_all[:, :, :, K - 1]
                elif use_fused:
                    s_all = pool_s.tile([128, NG, K, K], F32, tag="sall")
                    for g in range(NG):
                        nc.vector._custom_dve(
                            segmax,
                            out=s_all[:, g, :, :],
                            in0=mp[:, g, :].unsqueeze(1).broadcast_to([128, K, K]),
                            in1=trepf3,
                        )
                    lpart = s_all[:, :, :, K - 1]
                else:
                    s_all = pool_s.tile([128, NG, K, K], F32, tag="sall")
                    nc.vector.tensor_tensor(
                        s_all[:],
                        mp.unsqueeze(2).broadcast_to([128, NG, K, K]),
                        trepf3.unsqueeze(1).broadcast_to([128, NG, K, K]),
                        mybir.AluOpType.add,
                    )
                    l_scr = pool_l.tile([128, NG, K], F32, tag="lscr")
                    nc.vector.tensor_reduce(
                        l_scr[:], s_all[:], mybir.AxisListType.X, mybir.AluOpType.max
                    )
                    lpart = l_scr[:]
                adds.tensor_tensor(
                    m_c[:, pos, :, :], e_sb[:, :, pos, :], lpart, mybir.AluOpType.add
                )
            for g in range(NG):
                nc.sync.dma_start(
                    m_dram[i0 : i0 + CH, g * 128 : (g + 1) * 128, :].rearrange(
                        "i p k -> p i k"
                    ),
                    m_c[:, :, g, :],
                )
            m_prev = m_c

    # ---------------- Phase B: backward scan + labels ----------------
    with (
        tc.tile_pool(name="ein", bufs=3) as pool_ein,
        tc.tile_pool(name="min", bufs=3) as pool_min,
        tc.tile_pool(name="tch", bufs=2) as pool_t,
        tc.tile_pool(name="s2", bufs=3) as pool_s2,
        tc.tile_pool(name="gpp", bufs=3) as pool_g,
        tc.tile_pool(name="bulk", bufs=2) as pool_bulk,
    ):
        G = None
        for c in reversed(range(NCH)):
            i0 = c * CH
            e_in = pool_ein.tile([128, CH, NG, K], F32, tag="ein")
            m_in = pool_min.tile([128, CH, NG, K], F32, tag="min")
            for g in range(NG):
                nc.sync.dma_start(
                    e_in[:, :, g, :],
                    e_dram[i0 : i0 + CH, g * 128 : (g + 1) * 128, :].rearrange(
                        "i p k -> p i k"
                    ),
                )
                nc.sync.dma_start(
                    m_in[:, :, g, :],
                    m_dram[i0 : i0 + CH, g * 128 : (g + 1) * 128, :].rearrange(
                        "i p k -> p i k"
                    ),
                )
            t_c = pool_t.tile([128, CH, NG, K], F32, tag="tch")
            for pos in reversed(range(CH)):
                i = i0 + pos
                if i == L - 1:
                    G = pool_g.tile([128, NG, K], F32, tag="g")
                    nc.vector.tensor_copy(G[:], e_in[:, pos, :, :])
                    nc.vector.tensor_copy(t_c[:, pos, :, :], m_in[:, pos, :, :])
                    continue
                s2 = pool_s2.tile([128, NG, K, K], F32, tag="s2")
                if use_fused == "fuse4":
                    _custom_dve_raw(
                        nc.vector,
                        segmax,
                        out=s2[:],
                        in0=G[:].unsqueeze(2).broadcast_to([128, NG, K, K]),
                        in1=trepg3.unsqueeze(1).broadcast_to([128, NG, K, K]),
                    )
                    maxpart = s2[:, :, :, K - 1]
                elif use_fused:
                    for g in range(NG):
                        nc.vector._custom_dve(
                            segmax,
                            out=s2[:, g, :, :],
                            in0=G[:, g, :].unsqueeze(1).broadcast_to([128, K, K]),
                            in1=trepg3,
                        )
                    maxpart = s2[:, :, :, K - 1]
                else:
                    nc.vector.tensor_tensor(
                        s2[:],
                        G[:].unsqueeze(2).broadcast_to([128, NG, K, K]),
                        trepg3.unsqueeze(1).broadcast_to([128, NG, K, K]),
                        mybir.AluOpType.add,
                    )
                    mp_scr = pool_g.tile([128, NG, K], F32, tag="mpart")
                    nc.vector.tensor_reduce(
                        mp_scr[:], s2[:], mybir.AxisListType.X, mybir.AluOpType.max
                    )
                    maxpart = mp_scr[:]
                Gn = pool_g.tile([128, NG, K], F32, tag="g")
                gadd_eng = nc.gpsimd if adds_engine == "gadd_gpsimd" else adds
                gadd_eng.tensor_tensor(
                    Gn[:], e_in[:, pos, :, :], maxpart, mybir.AluOpType.add
                )
                # t feeds only the end-of-chunk bulk argmax (latency-tolerant)
                # -> run it on the otherwise-idle GPSIMD to shorten the DVE
                # stream, which real HW shows is per-op-overhead sensitive.
                nc.gpsimd.tensor_tensor(
                    t_c[:, pos, :, :], m_in[:, pos, :, :], maxpart, mybir.AluOpType.add
                )
                G = Gn
            # bulk argmax over y for this chunk
            tmax = pool_bulk.tile([128, CH, NG], F32, tag="tmax")
            nc.vector.tensor_reduce(
                tmax[:], t_c[:], mybir.AxisListType.X, mybir.AluOpType.max
            )
            mask = pool_bulk.tile([128, CH, NG, K], F32, tag="mask")
            nc.vector.tensor_tensor(
                mask[:],
                t_c[:],
                tmax[:].unsqueeze(3).broadcast_to([128, CH, NG, K]),
                mybir.AluOpType.is_equal,
            )
            cand = pool_bulk.tile([128, CH, NG, K], F32, tag="mask")
            nc.vector.tensor_tensor(
                cand[:],
                mask[:],
                revy[:].unsqueeze(1).unsqueeze(1).broadcast_to([128, CH, NG, K]),
                mybir.AluOpType.mult,
            )
            rc = pool_bulk.tile([128, CH, NG], F32, tag="tmax")
            nc.vector.tensor_reduce(
                rc[:], cand[:], mybir.AxisListType.X, mybir.AluOpType.max
            )
            lblf = pool_bulk.tile([128, CH, NG], F32, tag="lblf")
            nc.vector.tensor_scalar(
                lblf[:], rc[:], -1.0, 26.0, mybir.AluOpType.mult, mybir.AluOpType.add
            )
            nc.vector.tensor_copy(
                labels_sb[:, :, i0 : i0 + CH].transpose([0, 2, 1]), lblf[:]
            )
    for g in range(NG):
        nc.sync.dma_start(labels_out[g * 128 : (g + 1) * 128, :], labels_sb[:, g, :])


# ---------------------------------------------------------------------------
# Host-side driver
# ---------------------------------------------------------------------------


def _host_consts(W, T):
    K_, D_ = W.shape
    assert (K_, D_) == (K, D)
    wt = np.ascontiguousarray(W.T).astype(np.float32)  # [128, 26]
    trepf = np.tile(np.ascontiguousarray(T.T).reshape(1, -1), (128, 1)).astype(np.float32)
    trepg = np.tile(np.ascontiguousarray(T).reshape(1, -1), (128, 1)).astype(np.float32)
    ident = np.eye(128, dtype=np.float32)
    revy = np.tile((26.0 - np.arange(K, dtype=np.float32))[None], (128, 1))
    return {"wt": wt, "trepf": trepf, "trepg": trepg, "ident": ident, "revy": revy}


_prog_cache = {}


def build_program(L=512, use_fused=True, adds_engine="vector"):
    key = (L, use_fused, adds_engine)
    if key in _prog_cache:
        return _prog_cache[key]
    from contextlib import ExitStack

    nc = bacc.Bacc("TRN2", target_bir_lowering=False, debug=False)
    in_aps = {
        "X": nc.dram_tensor("X", [NG * 128, L, D], F32, kind="ExternalInput").ap(),
        "wt": nc.dram_tensor("wt", [128, K], F32, kind="ExternalInput").ap(),
        "trepf": nc.dram_tensor("trepf", [128, K * K], F32, kind="ExternalInput").ap(),
        "trepg": nc.dram_tensor("trepg", [128, K * K], F32, kind="ExternalInput").ap(),
        "ident": nc.dram_tensor("ident", [128, 128], F32, kind="ExternalInput").ap(),
        "revy": nc.dram_tensor("revy", [128, K], F32, kind="ExternalInput").ap(),
    }
    out_aps = {
        "labels": nc.dram_tensor("labels", [NG * 128, L], I32, kind="ExternalOutput").ap()
    }
    with tile.TileContext(nc) as tc:
        with ExitStack() as ctx:
            build_crf_kernel(
                ctx, tc, out_aps, in_aps, L=L, use_fused=use_fused, adds_engine=adds_engine
            )
    nc.compile()
    _prog_cache[key] = nc
    return nc


def kernel(X, W, T):
    X = np.ascontiguousarray(X, dtype=np.float32)
    W = np.ascontiguousarray(W, dtype=np.float32)
    T = np.ascontiguousarray(T, dtype=np.float32)
    B, L, D_ = X.shape
    wpc = B // NUM_CORES
    assert wpc == NG * 128 and D_ == D

    consts = _host_consts(W, T)
    nc = build_program(L=L, use_fused=True)
    in_maps = []
    for c in range(NUM_CORES):
        m = {"X": X[c * wpc : (c + 1) * wpc]}
        m.update(consts)
        in_maps.append(m)
    res = run_bass_kernel_spmd(nc, in_maps, list(range(NUM_CORES)))
    out = np.concatenate([r["labels"] for r in res.results], axis=0)
    return out.astype(np.int32)


if __name__ == "__main__":
    # smoke test at small L against a numpy reference
    rng = np.random.default_rng(0)
    L = 64
    X = rng.standard_normal((NUM_CORES * NG * 128, L, D)).astype(np.float32)
    W = rng.standard_normal((K, D)).astype(np.float32)
    T = rng.standard_normal((K, K)).astype(np.float32)
    lab = kernel(X, W, T)
    print(lab.shape, lab.dtype, lab[:2, :8])



# revision 9
# speedup vs baseline: 1.1499x; 1.1499x over previous
"""CRF Viterbi decode kernel for Trainium2 (8 NeuronCores, pure data parallel).

Problem: X [4096, 512, 128] f32, W [26, 128], T [26, 26]
  e = einsum('bld,kd->blk', X, W)
  per word: Viterbi max-sum scan over L=512 with transition T, backtrace,
  output int32 labels [4096, 512].

Strategy (per core, 512 words):
  - shard batch across 8 cores (512 words each), replicate W/T.
  - words on partitions: 4 groups of 128 words.
  - emissions on PE: per (group, position) transpose X tile [128w,128d] ->
    [128d,128w] (PE transpose), matmul with W^T -> e [128w, 26] in PSUM,
    batch-copy to SBUF, spill e to DRAM (needed again by backward scan).
  - forward scan (l) and backward scan (G) instead of backtrace:
      l[i,y] = max_j(e[i-1,j] + T[j,y] + l[i-1,j])     (reference's scan)
      G[i,j] = e[i,j] + max_y(T[j,y] + G[i+1,y]),  G[L-1] = e[L-1]
      label[i] = argmax_y(l[i,y] + G[i,y])
    We keep m = e + l (the quantity the scan consumes anyway); then
    t[i] = m[i] + maxpart[i] where G[i] = e[i] + maxpart[i].
  - the inner max-plus step uses a hand-built custom DVE op SEG_MAX_ADD_ANT:
    out[p,s,k] = running max over k within page s of (in0+in1), so
    out[:, y, 25] = max_j(m[j] + T[j,y]) in ONE pass (fused add+max with
    per-page reset via the SUB_DIM_DONE step-state, like TENSOR_PAGED_MASK).
  - m is spilled to DRAM during the forward pass, streamed back (with e)
    during the backward pass; labels via bulk masked argmax per chunk.
"""

import os
import sys

for _p in ("/opt/trn_rl_repo", "/root/.axon_site/_ro/trn_rl_repo"):
    if os.path.isdir(_p) and _p not in sys.path:
        sys.path.append(_p)

import dataclasses
import numpy as np

import concourse.bass as bass
import concourse.tile as tile
from concourse import bacc, mybir
from concourse.bass_utils import run_bass_kernel_spmd

F32 = mybir.dt.float32
I32 = mybir.dt.int32

NUM_CORES = 8
K = 26
D = 128
NG = 4  # word groups of 128 per core

# ---------------------------------------------------------------------------
# Custom DVE op: segmented (per-page) running max of (Src0 + Src1).
# ---------------------------------------------------------------------------

_SEGMAX_NAME = "SEG_MAX_ADD_ANT"
_segmax_op = None


def _segmax_reference(in0, in1, c0, c1, c2):
    z = np.asarray(in0, np.float32) + np.asarray(in1, np.float32)
    return np.maximum.accumulate(z, axis=-1)


def _build_segmax_uops(ver):
    from concourse import dve_spec as ds
    from concourse.dve_spec import AluOp, Spec, Src0, Src1, Trigger, scan

    spec = Spec(body=scan(AluOp.MAX, Src0 + Src1), reference=_segmax_reference)
    ds._validate_body(spec, ver)
    spec2 = ds._hoist_stream_invariant_ops(spec)
    scans = ds._collect(spec2.body, ds.Scan)
    placement = ds._build_placement(spec2, scans, ds.N_STAGES[ver], ds.N_LANES[ver])
    base = ds._build_state_machine(spec2, scans, [], placement)
    assert len(base) == 2  # [seed, steady]
    d = placement.node_stage[scans[0]]
    steady_stage = placement.pipeline[d]
    seed = base[0]
    steady = dataclasses.replace(
        base[1],
        trigger=(Trigger.SRC_TENSOR_DONE, Trigger.SUB_DIM_DONE, Trigger.NONE),
        next=(0, 2, 0),
    )
    # Boundary element: recombine against -FLT_MAX instead of the running
    # value -> the fold restarts at each page, exactly the PageIdx step-state
    # shape with the combine kept.
    step = dataclasses.replace(
        base[1],
        trigger=(Trigger.SRC_TENSOR_DONE, Trigger.SUB_DIM_DONE, Trigger.COUNT),
        next=(0, 2, 1),
        repeat=1,
        overrides={d: ds._Stage(AluOp.MAX, ds.MaxNeg, steady_stage.b)},
    )
    uops = [ds._assemble(s) for s in (seed, steady, step)]
    for u in uops:
        u.validate(ver)
    return spec, uops


def get_segmax_op():
    """Build + register the custom op in the dve_ops registries (idempotent)."""
    global _segmax_op
    if _segmax_op is not None:
        return _segmax_op
    from concourse import dve_ops
    from concourse.dve_ops import OPS, CUSTOM_DVE_SPECS, _SUB_OPCODE_FOR_NAME, DveOp
    from concourse.dve_uop import DveOpSpec

    @dataclasses.dataclass(frozen=True)
    class _HandBuiltDveOp(DveOp):
        def compile(self, ver):
            key = (self.name, ver)
            if (r := dve_ops._COMPILE_CACHE.get(key)) is not None:
                return r
            from concourse.dve_ops import get_dve_sub_opcode

            _, uops = _build_segmax_uops(ver)
            result = DveOpSpec(
                name=self.name,
                opcode=get_dve_sub_opcode(self.name),
                uops=uops,
                rd1_en=True,
            )
            dve_ops._COMPILE_CACHE[key] = result
            return result

    spec, _ = _build_segmax_uops("v3")
    op = _HandBuiltDveOp(_SEGMAX_NAME, spec, subdim=True, uops_sha={})
    if _SEGMAX_NAME not in _SUB_OPCODE_FOR_NAME:
        OPS.append(op)
        CUSTOM_DVE_SPECS[_SEGMAX_NAME] = spec
        _SUB_OPCODE_FOR_NAME[_SEGMAX_NAME] = 1 + len(OPS) - 1
        assert _SUB_OPCODE_FOR_NAME[_SEGMAX_NAME] < 0x20
    _segmax_op = op
    return op


def _custom_dve_raw(vec, op, out, in0, in1):
    """_custom_dve minus the <=2-free-dim assert: emit one InstCustomDveAnt
    over 3-free-dim APs so all 4 word groups ride in a single instruction.
    SUB_DIM_DONE fires on every innermost-dim wrap, so per-(g,y)-page reset
    semantics are unchanged."""
    import concourse.bass_isa as bass_isa
    from concourse.bass import MemorySpace
    from concourse.dve_ops import get_dve_sub_opcode
    from concourse.dve_table_gen import dve_ver_for
    from concourse import mybir as mb

    nc = vec.bass
    if op.name not in nc.m.ant_custom_dve_ops:
        nc.m.ant_custom_dve_ops = sorted({*nc.m.ant_custom_dve_ops, op.name})
    compiled = op.compile(dve_ver_for(nc.trn_type))
    shape = bass_isa.CustomDveShape.STT  # in1 has 2+ free dims
    isa_opcode = nc.isa.Opcode[
        f"NEURON_ISA_TPB_OPCODE_CUSTOM_DVE_ANT_{shape.slot()}"
    ].value
    imm = mb.ImmediateValue(dtype=mb.dt.float32, value=0.0)
    return vec.add_instruction(
        bass_isa.InstCustomDveAnt(
            name=nc.get_next_instruction_name(),
            op_name=op.name,
            rd1_en=compiled.rd1_en,
            subdim=0x02 if op.subdim else 0,
            imm2=0.0,
            shape=shape,
            row=get_dve_sub_opcode(op.name),
            isa_opcode=isa_opcode,
            ins=[
                vec.lower_ap(in0, for_isa=True, opt=False),
                vec.lower_ap(in1, for_isa=True, opt=False),
                imm,
                mb.ImmediateValue(dtype=mb.dt.float32, value=0.0),
            ],
            outs=[vec.lower_ap(out, for_isa=True, opt=False)],
        )
    )


# ---------------------------------------------------------------------------
# Program builder (backtrace variant)
# ---------------------------------------------------------------------------


def build_crf_kernel_bt(ctx, tc, out_aps, in_aps, L=512, adds_engine="vector"):
    """Forward max-plus scan (custom DVE segmax) + true Viterbi backtrace.

    Backward phase per word-group chain (words on partitions):
      y_{i-1} = argmax_j( m[i-1, j] + T[j, y_i] )   (== reference bp)
    The per-word T column T[:, y_i] is selected with a PE matmul against the
    one-hot of y_i (contraction over partitions after a PE transpose), so the
    backward step is 4 tiny DVE ops instead of a 676-element max-plus scan.
    First-occurrence argmax is exact: r = max((26-j)*onehot) -> j* = 26-r,
    and the clean one-hot is regenerated from r.
    """
    nc = tc.nc
    X = in_aps["X"]
    labels_out = out_aps["labels"]
    adds = nc.gpsimd if adds_engine == "gpsimd" else nc.vector

    KK = K * K
    CH = min(32, L)
    assert L % CH == 0
    NCH = L // CH
    XCH = min(8, L)
    EPS = min(16, CH)

    segmax = get_segmax_op()

    # DRAM scratch for m only (e is not needed by the backward phase).
    m_dram = nc.dram_tensor(f"m_scr_{L}", [L, NG * 128, K], F32).ap()

    cpool = ctx.enter_context(tc.tile_pool(name="consts", bufs=1))
    trepf = cpool.tile([128, KK], F32, tag="trepf")
    wt = cpool.tile([128, K], F32, tag="wt")
    ident = cpool.tile([128, 128], F32, tag="ident")
    zc = cpool.tile([128, K], F32, tag="zc")  # zeros
    tmm2 = cpool.tile([128, K + 1], F32, tag="tmm2")  # rows y: [T[:, y] | y]
    nc.sync.dma_start(trepf[:], in_aps["trepf"])
    nc.sync.dma_start(wt[:], in_aps["wt"])
    nc.sync.dma_start(ident[:], in_aps["ident"])
    nc.sync.dma_start(zc[:], in_aps["zc"])
    nc.sync.dma_start(tmm2[:], in_aps["tmm2"])
    trepf3 = trepf[:].rearrange("p (y j) -> p y j", j=K)

    labels_pool = ctx.enter_context(tc.tile_pool(name="labels", bufs=1))
    lblr = labels_pool.tile([128, NG, L], F32)  # y per position
    labels_sb = labels_pool.tile([128, NG, L], I32)
    # per-group step-function tiles; col 0 is a permanent zero pad so the
    # shifted subtract yields the first-occurrence one-hot in one op.
    steps = [
        labels_pool.tile([128, K + 1], F32, name=f"stepfn{g}") for g in range(NG)
    ]

    # ---------------- Phase A: emissions + forward scan ----------------
    with (
        tc.tile_pool(name="xstage", bufs=2) as pool_x,
        tc.tile_pool(name="esb", bufs=3) as pool_esb,
        tc.tile_pool(name="mout", bufs=3) as pool_m,
        tc.tile_pool(name="sall", bufs=3) as pool_s,
        tc.tile_pool(name="xt_ps", bufs=3, space="PSUM") as pool_xtps,
        tc.tile_pool(name="eps", bufs=1, space="PSUM") as pool_eps,
        tc.tile_pool(name="xtsb", bufs=3) as pool_xtsb,
    ):
        m_prev = None
        for c in range(NCH):
            i0 = c * CH
            e_sb = pool_esb.tile([128, NG, CH, K], F32, tag="esb")
            m_c = pool_m.tile([128, CH, NG, K], F32, tag="mout")
            for g in range(NG):
                for h in range(CH // EPS):
                    eps = pool_eps.tile([128, EPS * K], F32, tag=f"eps{g}")
                    for ii in range(EPS):
                        pos = h * EPS + ii
                        sc, si = divmod(pos, XCH)
                        if si == 0:
                            xs = pool_x.tile([128, XCH * D], F32, tag=f"xs{g}")
                            nc.sync.dma_start(
                                xs[:],
                                X[g * 128 : (g + 1) * 128, i0 + sc * XCH : i0 + (sc + 1) * XCH, :],
                            )
                            xs3 = xs[:].rearrange("p (i d) -> p i d", d=D)
                        xt_ps = pool_xtps.tile([128, 128], F32, tag="xt")
                        nc.tensor.transpose(xt_ps[:], xs3[:, si, :], ident[:])
                        xt_sb = pool_xtsb.tile([128, 128], F32, tag="xt_sb")
                        nc.scalar.copy(xt_sb[:], xt_ps[:])
                        nc.tensor.matmul(
                            eps[:, ii * K : (ii + 1) * K],
                            lhsT=xt_sb[:],
                            rhs=wt[:],
                            start=True,
                            stop=True,
                            skip_group_check=True,
                        )
                    nc.scalar.copy(
                        e_sb[:, g, h * EPS : (h + 1) * EPS, :],
                        eps[:].rearrange("p (i k) -> p i k", k=K),
                    )
            # forward scan over this chunk
            for pos in range(CH):
                i = i0 + pos
                if i == 0:
                    nc.vector.tensor_copy(m_c[:, 0, :, :], e_sb[:, :, 0, :])
                    continue
                mp = m_prev[:, CH - 1, :, :] if pos == 0 else m_c[:, pos - 1, :, :]
                s_all = pool_s.tile([128, NG, K, K], F32, tag="sall")
                for g in range(NG):
                    nc.vector._custom_dve(
                        segmax,
                        out=s_all[:, g, :, :],
                        in0=mp[:, g, :].unsqueeze(1).broadcast_to([128, K, K]),
                        in1=trepf3,
                    )
                adds.tensor_tensor(
                    m_c[:, pos, :, :],
                    e_sb[:, :, pos, :],
                    s_all[:, :, :, K - 1],
                    mybir.AluOpType.add,
                )
            for g in range(NG):
                nc.sync.dma_start(
                    m_dram[i0 : i0 + CH, g * 128 : (g + 1) * 128, :].rearrange(
                        "i p k -> p i k"
                    ),
                    m_c[:, :, g, :],
                )
            m_prev = m_c

    # ---------------- Phase B: backtrace ----------------
    with (
        tc.tile_pool(name="min", bufs=2) as pool_min,
        tc.tile_pool(name="ot_ps", bufs=1, space="PSUM") as pool_otps,
        tc.tile_pool(name="mm_ps", bufs=1, space="PSUM") as pool_mmps,
        tc.tile_pool(name="ot_sb", bufs=2) as pool_otsb,
        tc.tile_pool(name="bscr", bufs=2) as pool_bs,
        tc.tile_pool(name="oh", bufs=2) as pool_oh,
    ):
        def load_chunk(c):
            m_in = pool_min.tile([128, CH, NG, K], F32, tag="min")
            for g in range(NG):
                nc.sync.dma_start(
                    m_in[:, :, g, :],
                    m_dram[c * CH : (c + 1) * CH, g * 128 : (g + 1) * 128, :].rearrange(
                        "i p k -> p i k"
                    ),
                )
            return m_in

        for g in range(NG):
            nc.vector.memset(steps[g][:], 0.0)

        def argmax_step(g, m_slice, tcol_ap):
            """srun = segmax(m + tcol); step fn; one-hot o_g (first argmax)."""
            srun = pool_bs.tile([128, K], F32, tag=f"sr{g}")
            nc.vector._custom_dve(
                segmax,
                out=srun[:].rearrange("p (a k) -> p a k", a=1),
                in0=m_slice.rearrange("p (a k) -> p a k", a=1)
                if len(m_slice.shape) == 2
                else m_slice.unsqueeze(1),
                in1=tcol_ap.rearrange("p (a k) -> p a k", a=1),
            )
            nc.vector.tensor_tensor(
                steps[g][:, 1 : K + 1],
                srun[:],
                srun[:, K - 1 : K].broadcast_to([128, K]),
                mybir.AluOpType.is_equal,
            )
            o_g = pool_oh.tile([128, K], F32, tag=f"o{g}")
            nc.vector.tensor_tensor(
                o_g[:],
                steps[g][:, 1 : K + 1],
                steps[g][:, 0:K],
                mybir.AluOpType.subtract,
            )
            return o_g

        def select_mm(g, o_g):
            """PE-select: mm_ps = [T[:, y] | y] for each word's one-hot y."""
            ot_ps = pool_otps.tile([K, 128], F32, tag=f"ot{g}")
            nc.tensor.transpose(ot_ps[:], o_g[:], ident[:])
            ot_sb = pool_otsb.tile([K, 128], F32, tag=f"otsb{g}")
            nc.scalar.copy(ot_sb[:], ot_ps[:])
            mm_ps = pool_mmps.tile([128, K + 1], F32, tag=f"mm{g}")
            nc.tensor.matmul(
                mm_ps[:],
                lhsT=ot_sb[:],
                rhs=tmm2[:K, :],
                start=True,
                stop=True,
                skip_group_check=True,
            )
            return mm_ps

        m_in = load_chunk(NCH - 1)
        # init at i = L-1: o = one-hot of first argmax of m[L-1]
        o_cur = [
            argmax_step(g, m_in[:, CH - 1, g, :], zc[:]) for g in range(NG)
        ]

        for c in reversed(range(NCH)):
            m_next = load_chunk(c - 1) if c > 0 else None
            h_hi = c * CH + CH - 2 if c == NCH - 1 else c * CH + CH - 1
            for h in range(h_hi, c * CH - 1, -1):
                pos = h - c * CH
                for g in range(NG):
                    mm_ps = select_mm(g, o_cur[g])
                    # label for position h+1 (the y encoded in o_cur)
                    nc.vector.tensor_copy(
                        lblr[:, g, h + 1 : h + 2], mm_ps[:, K : K + 1]
                    )
                    o_cur[g] = argmax_step(
                        g, m_in[:, pos, g, :], mm_ps[:, 0:K]
                    )
            if m_next is not None:
                m_in = m_next

        # tail: extract label for position 0
        for g in range(NG):
            mm_ps = select_mm(g, o_cur[g])
            nc.vector.tensor_copy(lblr[:, g, 0:1], mm_ps[:, K : K + 1])

    nc.vector.tensor_copy(labels_sb[:], lblr[:])
    for g in range(NG):
        nc.sync.dma_start(labels_out[g * 128 : (g + 1) * 128, :], labels_sb[:, g, :])


# ---------------------------------------------------------------------------
# Program builder (original forward/backward-scan variant)
# ---------------------------------------------------------------------------


def build_crf_kernel(ctx, tc, out_aps, in_aps, L=512, use_fused=True, adds_engine="vector"):
    """Emit the per-core CRF decode program.

    in_aps: dict with DRAM APs: X [NG*128, L, D], wt [128, K] (= W^T),
      trepf [128, K*K] (T^T flat: [y*26+j] = T[j,y]),
      trepg [128, K*K] (T flat:  [j*26+y] = T[j,y]),
      ident [128, 128], revy [128, K] (= 26 - y).
    out_aps: dict with labels [NG*128, L] int32.
    """
    nc = tc.nc
    X = in_aps["X"]
    labels_out = out_aps["labels"]
    adds = nc.gpsimd if adds_engine == "gpsimd" else nc.vector

    KK = K * K
    CH = min(32, L)  # scan/spill chunk
    assert L % CH == 0
    NCH = L // CH
    XCH = min(8, L)  # X staging sub-chunk
    EPS = min(16, CH)  # emissions per PSUM bank tile (16*26=416 fp32 <= 512)

    segmax = get_segmax_op() if use_fused else None

    # DRAM scratch for e and m, layout [i, w, k] (i-major so a chunk of
    # positions for all words is one contiguous region).
    e_dram = nc.dram_tensor(f"e_scr_{L}", [L, NG * 128, K], F32).ap()
    m_dram = nc.dram_tensor(f"m_scr_{L}", [L, NG * 128, K], F32).ap()

    cpool = ctx.enter_context(tc.tile_pool(name="consts", bufs=1))
    trepf = cpool.tile([128, KK], F32, tag="trepf")
    trepg = cpool.tile([128, KK], F32, tag="trepg")
    wt = cpool.tile([128, K], F32, tag="wt")
    ident = cpool.tile([128, 128], F32, tag="ident")
    revy = cpool.tile([128, K], F32, tag="revy")
    nc.sync.dma_start(trepf[:], in_aps["trepf"])
    nc.sync.dma_start(trepg[:], in_aps["trepg"])
    nc.sync.dma_start(wt[:], in_aps["wt"])
    nc.sync.dma_start(ident[:], in_aps["ident"])
    nc.sync.dma_start(revy[:], in_aps["revy"])
    trepf3 = trepf[:].rearrange("p (y j) -> p y j", j=K)
    trepg3 = trepg[:].rearrange("p (j y) -> p j y", y=K)

    labels_pool = ctx.enter_context(tc.tile_pool(name="labels", bufs=1))
    labels_sb = labels_pool.tile([128, NG, L], I32)

    # ---------------- Phase A: emissions + forward scan ----------------
    with (
        tc.tile_pool(name="xstage", bufs=2) as pool_x,
        tc.tile_pool(name="esb", bufs=3) as pool_esb,
        tc.tile_pool(name="mout", bufs=3) as pool_m,
        tc.tile_pool(name="sall", bufs=3) as pool_s,
        tc.tile_pool(name="lscr", bufs=2) as pool_l,
        tc.tile_pool(name="xt_ps", bufs=3, space="PSUM") as pool_xtps,
        tc.tile_pool(name="eps", bufs=1, space="PSUM") as pool_eps,
        tc.tile_pool(name="xtsb", bufs=3) as pool_xtsb,
    ):
        m_prev = None
        for c in range(NCH):
            i0 = c * CH
            e_sb = pool_esb.tile([128, NG, CH, K], F32, tag="esb")
            m_c = pool_m.tile([128, CH, NG, K], F32, tag="mout")
            for g in range(NG):
                # emissions for (group g, positions i0..i0+CH)
                for h in range(CH // EPS):
                    eps = pool_eps.tile([128, EPS * K], F32, tag=f"eps{g}")
                    for ii in range(EPS):
                        pos = h * EPS + ii
                        sc, si = divmod(pos, XCH)
                        if si == 0:
                            xs = pool_x.tile([128, XCH * D], F32, tag=f"xs{g}")
                            nc.sync.dma_start(
                                xs[:],
                                X[g * 128 : (g + 1) * 128, i0 + sc * XCH : i0 + (sc + 1) * XCH, :],
                            )
                            xs3 = xs[:].rearrange("p (i d) -> p i d", d=D)
                        xt_ps = pool_xtps.tile([128, 128], F32, tag="xt")
                        nc.tensor.transpose(xt_ps[:], xs3[:, si, :], ident[:])
                        xt_sb = pool_xtsb.tile([128, 128], F32, tag="xt_sb")
                        nc.scalar.copy(xt_sb[:], xt_ps[:])
                        nc.tensor.matmul(
                            eps[:, ii * K : (ii + 1) * K],
                            lhsT=xt_sb[:],
                            rhs=wt[:],
                            start=True,
                            stop=True,
                            skip_group_check=True,
                        )
                    nc.scalar.copy(
                        e_sb[:, g, h * EPS : (h + 1) * EPS, :],
                        eps[:].rearrange("p (i k) -> p i k", k=K),
                    )
            for g in range(NG):
                nc.sync.dma_start(
                    e_dram[i0 : i0 + CH, g * 128 : (g + 1) * 128, :].rearrange(
                        "i p k -> p i k"
                    ),
                    e_sb[:, g, :, :],
                )
            # forward scan over this chunk
            for pos in range(CH):
                i = i0 + pos
                if i == 0:
                    nc.vector.tensor_copy(m_c[:, 0, :, :], e_sb[:, :, 0, :])
                    continue
                mp = m_prev[:, CH - 1, :, :] if pos == 0 else m_c[:, pos - 1, :, :]
                if use_fused == "fuse4":
                    s_all = pool_s.tile([128, NG, K, K], F32, tag="sall")
                    _custom_dve_raw(
                        nc.vector,
                        segmax,
                        out=s_all[:],
                        in0=mp.unsqueeze(2).broadcast_to([128, NG, K, K]),
                        in1=trepf3.unsqueeze(1).broadcast_to([128, NG, K, K]),
                    )
                    lpart = s_all[:, :, :, K - 1]
                elif use_fused:
                    s_all = pool_s.tile([128, NG, K, K], F32, tag="sall")
                    for g in range(NG):
                        nc.vector._custom_dve(
                            segmax,
                            out=s_all[:, g, :, :],
                            in0=mp[:, g, :].unsqueeze(1).broadcast_to([128, K, K]),
                            in1=trepf3,
                        )
                    lpart = s_all[:, :, :, K - 1]
                else:
                    s_all = pool_s.tile([128, NG, K, K], F32, tag="sall")
                    nc.vector.tensor_tensor(
                        s_all[:],
                        mp.unsqueeze(2).broadcast_to([128, NG, K, K]),
                        trepf3.unsqueeze(1).broadcast_to([128, NG, K, K]),
                        mybir.AluOpType.add,
                    )
                    l_scr = pool_l.tile([128, NG, K], F32, tag="lscr")
                    nc.vector.tensor_reduce(
                        l_scr[:], s_all[:], mybir.AxisListType.X, mybir.AluOpType.max
                    )
                    lpart = l_scr[:]
                adds.tensor_tensor(
                    m_c[:, pos, :, :], e_sb[:, :, pos, :], lpart, mybir.AluOpType.add
                )
            for g in range(NG):
                nc.sync.dma_start(
                    m_dram[i0 : i0 + CH, g * 128 : (g + 1) * 128, :].rearrange(
                        "i p k -> p i k"
                    ),
                    m_c[:, :, g, :],
                )
            m_prev = m_c

    # ---------------- Phase B: backward scan + labels ----------------
    with (
        tc.tile_pool(name="ein", bufs=3) as pool_ein,
        tc.tile_pool(name="min", bufs=3) as pool_min,
        tc.tile_pool(name="tch", bufs=2) as pool_t,
        tc.tile_pool(name="s2", bufs=3) as pool_s2,
        tc.tile_pool(name="gpp", bufs=3) as pool_g,
        tc.tile_pool(name="bulk", bufs=2) as pool_bulk,
    ):
        G = None
        for c in reversed(range(NCH)):
            i0 = c * CH
            e_in = pool_ein.tile([128, CH, NG, K], F32, tag="ein")
            m_in = pool_min.tile([128, CH, NG, K], F32, tag="min")
            for g in range(NG):
                nc.sync.dma_start(
                    e_in[:, :, g, :],
                    e_dram[i0 : i0 + CH, g * 128 : (g + 1) * 128, :].rearrange(
                        "i p k -> p i k"
                    ),
                )
                nc.sync.dma_start(
                    m_in[:, :, g, :],
                    m_dram[i0 : i0 + CH, g * 128 : (g + 1) * 128, :].rearrange(
                        "i p k -> p i k"
                    ),
                )
            t_c = pool_t.tile([128, CH, NG, K], F32, tag="tch")
            for pos in reversed(range(CH)):
                i = i0 + pos
                if i == L - 1:
                    G = pool_g.tile([128, NG, K], F32, tag="g")
                    nc.vector.tensor_copy(G[:], e_in[:, pos, :, :])
                    nc.vector.tensor_copy(t_c[:, pos, :, :], m_in[:, pos, :, :])
                    continue
                s2 = pool_s2.tile([128, NG, K, K], F32, tag="s2")
                if use_fused == "fuse4":
                    _custom_dve_raw(
                        nc.vector,
                        segmax,
                        out=s2[:],
                        in0=G[:].unsqueeze(2).broadcast_to([128, NG, K, K]),
                        in1=trepg3.unsqueeze(1).broadcast_to([128, NG, K, K]),
                    )
                    maxpart = s2[:, :, :, K - 1]
                elif use_fused:
                    for g in range(NG):
                        nc.vector._custom_dve(
                            segmax,
                            out=s2[:, g, :, :],
                            in0=G[:, g, :].unsqueeze(1).broadcast_to([128, K, K]),
                            in1=trepg3,
                        )
                    maxpart = s2[:, :, :, K - 1]
                else:
                    nc.vector.tensor_tensor(
                        s2[:],
                        G[:].unsqueeze(2).broadcast_to([128, NG, K, K]),
                        trepg3.unsqueeze(1).broadcast_to([128, NG, K, K]),
                        mybir.AluOpType.add,
                    )
                    mp_scr = pool_g.tile([128, NG, K], F32, tag="mpart")
                    nc.vector.tensor_reduce(
                        mp_scr[:], s2[:], mybir.AxisListType.X, mybir.AluOpType.max
                    )
                    maxpart = mp_scr[:]
                Gn = pool_g.tile([128, NG, K], F32, tag="g")
                gadd_eng = nc.gpsimd if adds_engine == "gadd_gpsimd" else adds
                gadd_eng.tensor_tensor(
                    Gn[:], e_in[:, pos, :, :], maxpart, mybir.AluOpType.add
                )
                # t feeds only the end-of-chunk bulk argmax (latency-tolerant)
                # -> run it on the otherwise-idle GPSIMD to shorten the DVE
                # stream, which real HW shows is per-op-overhead sensitive.
                nc.gpsimd.tensor_tensor(
                    t_c[:, pos, :, :], m_in[:, pos, :, :], maxpart, mybir.AluOpType.add
                )
                G = Gn
            # bulk argmax over y for this chunk
            tmax = pool_bulk.tile([128, CH, NG], F32, tag="tmax")
            nc.vector.tensor_reduce(
                tmax[:], t_c[:], mybir.AxisListType.X, mybir.AluOpType.max
            )
            mask = pool_bulk.tile([128, CH, NG, K], F32, tag="mask")
            nc.vector.tensor_tensor(
                mask[:],
                t_c[:],
                tmax[:].unsqueeze(3).broadcast_to([128, CH, NG, K]),
                mybir.AluOpType.is_equal,
            )
            cand = pool_bulk.tile([128, CH, NG, K], F32, tag="mask")
            nc.vector.tensor_tensor(
                cand[:],
                mask[:],
                revy[:].unsqueeze(1).unsqueeze(1).broadcast_to([128, CH, NG, K]),
                mybir.AluOpType.mult,
            )
            rc = pool_bulk.tile([128, CH, NG], F32, tag="tmax")
            nc.vector.tensor_reduce(
                rc[:], cand[:], mybir.AxisListType.X, mybir.AluOpType.max
            )
            lblf = pool_bulk.tile([128, CH, NG], F32, tag="lblf")
            nc.vector.tensor_scalar(
                lblf[:], rc[:], -1.0, 26.0, mybir.AluOpType.mult, mybir.AluOpType.add
            )
            nc.vector.tensor_copy(
                labels_sb[:, :, i0 : i0 + CH].transpose([0, 2, 1]), lblf[:]
            )
    for g in range(NG):
        nc.sync.dma_start(labels_out[g * 128 : (g + 1) * 128, :], labels_sb[:, g, :])


# ---------------------------------------------------------------------------
# Host-side driver
# ---------------------------------------------------------------------------


def _host_consts(W, T, variant="bt"):
    K_, D_ = W.shape
    assert (K_, D_) == (K, D)
    wt = np.ascontiguousarray(W.T).astype(np.float32)  # [128, 26]
    trepf = np.tile(np.ascontiguousarray(T.T).reshape(1, -1), (128, 1)).astype(np.float32)
    ident = np.eye(128, dtype=np.float32)
    revy = np.tile((26.0 - np.arange(K, dtype=np.float32))[None], (128, 1))
    if variant == "bt":
        out = {"wt": wt, "trepf": trepf, "ident": ident}
        tmm2 = np.zeros((128, K + 1), dtype=np.float32)
        tmm2[:K, :K] = np.ascontiguousarray(T.T)  # row y = T[:, y]
        tmm2[:K, K] = np.arange(K, dtype=np.float32)  # label column
        out["tmm2"] = tmm2
        out["zc"] = np.zeros((128, K), dtype=np.float32)
    else:
        out = {"wt": wt, "trepf": trepf, "ident": ident, "revy": revy}
        out["trepg"] = np.tile(
            np.ascontiguousarray(T).reshape(1, -1), (128, 1)
        ).astype(np.float32)
    return out


_prog_cache = {}


def build_program(L=512, use_fused=True, adds_engine="vector", variant="bt"):
    key = (L, use_fused, adds_engine, variant)
    if key in _prog_cache:
        return _prog_cache[key]
    from contextlib import ExitStack

    nc = bacc.Bacc("TRN2", target_bir_lowering=False, debug=False)
    in_aps = {
        "X": nc.dram_tensor("X", [NG * 128, L, D], F32, kind="ExternalInput").ap(),
        "wt": nc.dram_tensor("wt", [128, K], F32, kind="ExternalInput").ap(),
        "trepf": nc.dram_tensor("trepf", [128, K * K], F32, kind="ExternalInput").ap(),
        "ident": nc.dram_tensor("ident", [128, 128], F32, kind="ExternalInput").ap(),
    }
    if variant == "bt":
        in_aps["tmm2"] = nc.dram_tensor(
            "tmm2", [128, K + 1], F32, kind="ExternalInput"
        ).ap()
        in_aps["zc"] = nc.dram_tensor(
            "zc", [128, K], F32, kind="ExternalInput"
        ).ap()
    else:
        in_aps["revy"] = nc.dram_tensor(
            "revy", [128, K], F32, kind="ExternalInput"
        ).ap()
        in_aps["trepg"] = nc.dram_tensor(
            "trepg", [128, K * K], F32, kind="ExternalInput"
        ).ap()
    out_aps = {
        "labels": nc.dram_tensor("labels", [NG * 128, L], I32, kind="ExternalOutput").ap()
    }
    with tile.TileContext(nc) as tc:
        with ExitStack() as ctx:
            if variant == "bt":
                build_crf_kernel_bt(
                    ctx, tc, out_aps, in_aps, L=L, adds_engine=adds_engine
                )
            else:
                build_crf_kernel(
                    ctx, tc, out_aps, in_aps, L=L, use_fused=use_fused,
                    adds_engine=adds_engine,
                )
    nc.compile()
    _prog_cache[key] = nc
    return nc


def kernel(X, W, T):
    X = np.ascontiguousarray(X, dtype=np.float32)
    W = np.ascontiguousarray(W, dtype=np.float32)
    T = np.ascontiguousarray(T, dtype=np.float32)
    B, L, D_ = X.shape
    wpc = B // NUM_CORES
    assert wpc == NG * 128 and D_ == D

    consts = _host_consts(W, T, variant="bt")
    nc = build_program(L=L, variant="bt")
    in_maps = []
    for c in range(NUM_CORES):
        m = {"X": X[c * wpc : (c + 1) * wpc]}
        m.update(consts)
        in_maps.append(m)
    res = run_bass_kernel_spmd(nc, in_maps, list(range(NUM_CORES)))
    out = np.concatenate([r["labels"] for r in res.results], axis=0)
    return out.astype(np.int32)


if __name__ == "__main__":
    # smoke test at small L against a numpy reference
    rng = np.random.default_rng(0)
    L = 64
    X = rng.standard_normal((NUM_CORES * NG * 128, L, D)).astype(np.float32)
    W = rng.standard_normal((K, D)).astype(np.float32)
    T = rng.standard_normal((K, K)).astype(np.float32)
    lab = kernel(X, W, T)
    print(lab.shape, lab.dtype, lab[:2, :8])

